# revision 9
# baseline (speedup 1.0000x reference)
"""Trainium2 Bass kernel for nn_Attention_6897717477568 (QA-matching CNN+attention).

Sharding: pure data parallel, batch 64 -> 8 cores x 8 elems; parameters
replicated; one AllGather (768x8 f32 per core) feeds a replicated
BatchNorm-over-64 + heads tail.

Core algebraic trick: the dominant context-CNN over B*QL=2560 attention-
weighted sequences (attn_answer[q,a,:] = sent2[a,:]*cos[q,a], ~68 GFLOP
naive) is collapsed using linearity of the conv:
    Y[q,t,c] = sum_k cos[q,t+k] * Z[t+k,k,c],   Z = sent2 @ ctx_convW
Z is scattered into a block-banded matrix G with diagonal-stride DRAM DMAs
so the combination runs on the PE as dense matmuls, packing 3 batch elems
block-diagonally (cos rows as lhsT) per matmul.
"""
import os
import sys

sys.path.insert(0, '/opt/trn_rl_repo')

import numpy as np
from contextlib import ExitStack

import concourse.bass as bass
import concourse.tile as tile
from concourse import bacc, mybir
from concourse.masks import make_identity

f32 = mybir.dt.float32
i32 = mybir.dt.int32
AX = mybir.AxisListType
OP = mybir.AluOpType
AF = mybir.ActivationFunctionType

V, D = 50000, 300
C, K, H = 256, 3, 256
HLU, AH, NCLS = 512, 256, 2
B, QL, AL = 64, 40, 60
EPS_BN = 1e-5

NCORES = 8
BL = B // NCORES            # batch elems per core
PB = 64                     # padded token block per batch elem
TP = BL * PB                # 512 padded tokens per side
DCH = [(0, 128), (128, 128), (256, 44)]
TcH = 8                     # t-chunk size of banded G
NCH = 8                     # chunks cover t = 0..63 (valid 0..57)
RC = TcH * C                # 2048
GROUPS = [(0, 3), (3, 3), (6, 2)]
KC3 = K * C                 # 768


def _emit(nc, tc, ctx):
    sb = ctx.enter_context(tc.tile_pool(name="sb", bufs=1))
    wp = ctx.enter_context(tc.tile_pool(name="wp", bufs=1))
    pz = ctx.enter_context(tc.tile_pool(name="pz", bufs=2, space="PSUM"))
    ps = ctx.enter_context(tc.tile_pool(name="ps", bufs=2, space="PSUM"))
    pg = ctx.enter_context(tc.tile_pool(name="pg", bufs=2, space="PSUM"))
    dr = ctx.enter_context(tc.tile_pool(name="dr", bufs=1, space="DRAM"))

    # ---------------- inputs / outputs ----------------
    q_idx = nc.dram_tensor("q_idx", [TP], i32, kind="ExternalInput").ap()
    a_idx = nc.dram_tensor("a_idx", [TP], i32, kind="ExternalInput").ap()
    emb = nc.dram_tensor("emb", [V, D], f32, kind="ExternalInput").ap()
    smW = nc.dram_tensor("smW", [K, D, C], f32, kind="ExternalInput").ap()
    smb = nc.dram_tensor("smb", [C], f32, kind="ExternalInput").ap()
    smfcW = nc.dram_tensor("smfcW", [C, H], f32, kind="ExternalInput").ap()
    smfcb = nc.dram_tensor("smfcb", [H], f32, kind="ExternalInput").ap()
    ctxW = nc.dram_tensor("ctxW", [K, D, C], f32, kind="ExternalInput").ap()
    ctxb = nc.dram_tensor("ctxb", [C], f32, kind="ExternalInput").ap()
    ctxfcW = nc.dram_tensor("ctxfcW", [C, H], f32, kind="ExternalInput").ap()
    ctxfcb = nc.dram_tensor("ctxfcb", [H], f32, kind="ExternalInput").ap()
    attnW = nc.dram_tensor("attnW", [H + D, AH], f32, kind="ExternalInput").ap()
    attnb = nc.dram_tensor("attnb", [AH], f32, kind="ExternalInput").ap()
    probW = nc.dram_tensor("probW", [AH, 1], f32, kind="ExternalInput").ap()
    W1 = nc.dram_tensor("W1", [3 * H, HLU], f32, kind="ExternalInput").ap()
    b1 = nc.dram_tensor("b1", [HLU], f32, kind="ExternalInput").ap()
    gamma = nc.dram_tensor("gamma", [HLU], f32, kind="ExternalInput").ap()
    beta = nc.dram_tensor("beta", [HLU], f32, kind="ExternalInput").ap()
    W2 = nc.dram_tensor("W2", [HLU, NCLS], f32, kind="ExternalInput").ap()
    b2 = nc.dram_tensor("b2", [NCLS], f32, kind="ExternalInput").ap()

    preds_out = nc.dram_tensor("preds", [B, NCLS], f32, kind="ExternalOutput").ap()
    feat_out = nc.dram_tensor("feat", [B, HLU], f32, kind="ExternalOutput").ap()

    # ---------------- DRAM staging (pool tiles => dep-tracked) ----------------
    z_dram = dr.tile([TP + 8, KC3], f32, name="z_dram")
    g_dram = [dr.tile([NCH * 30 * RC], f32, name=f"g_dram{g}") for g in range(3)]
    bd_dram = [dr.tile([NCH * 30 * 120], f32, name=f"bd_dram{g}") for g in range(3)]
    cos_dram = dr.tile([BL * 68 * 40], f32, name="cos_dram")
    prob_dram = dr.tile([400], f32, name="prob_dram")
    pbd_dram = dr.tile([3 * 120 * 3], f32, name="pbd_dram")
    fc_dram = dr.tile([3 * H, BL], f32, name="fc_dram")
    fcall_dram = dr.tile([NCORES * 3 * H * BL], f32, addr_space="Shared",
                         name="fcall_dram")

    def dview(t, offset, dims):
        return bass.AP(tensor=t[:].tensor, offset=offset,
                       ap=[list(d) for d in dims])

    # ---------------- static setup ----------------
    ident = wp.tile([128, 128], f32, name="ident")
    make_identity(nc, ident[:])

    zt = wp.tile([128, RC], f32, name="zt")
    nc.gpsimd.memset(zt[:], 0.0)
    for g in range(3):
        n = NCH * 30 * RC
        step = 128 * RC
        for o in range(0, n, step):
            m = min(step, n - o)
            nc.sync.dma_start(dview(g_dram[g], o, [[RC, m // RC], [1, RC]]),
                              zt[0:m // RC, :])
        nc.sync.dma_start(dview(bd_dram[g], 0, [[1800, 16], [1, 1800]]),
                          zt[0:16, 0:1800])
    nc.sync.dma_start(dview(prob_dram, 0, [[400, 1], [1, 400]]), zt[0:1, 0:400])
    nc.sync.dma_start(dview(pbd_dram, 0, [[1080, 1], [1, 1080]]), zt[0:1, 0:1080])
    nc.sync.dma_start(z_dram[:][TP:TP + 8, :], zt[0:8, 0:KC3])

    def wtile(shape, src, name):
        t = wp.tile(shape, f32, name=name)
        nc.sync.dma_start(t[:], src)
        return t

    smWt, ctxWt = [], []
    for dc, (d0, dsz) in enumerate(DCH):
        smWt.append(wtile([dsz, KC3], smW[:, d0:d0 + dsz, :].transpose([1, 0, 2]),
                          f"smWt{dc}"))
        ctxWt.append(wtile([dsz, KC3], ctxW[:, d0:d0 + dsz, :].transpose([1, 0, 2]),
                           f"ctxWt{dc}"))
    smfcWt = [wtile([128, H], smfcW[c0:c0 + 128, :], f"smfcWt{c0}") for c0 in (0, 128)]
    ctxfcWt = [wtile([128, H], ctxfcW[c0:c0 + 128, :], f"ctxfcWt{c0}") for c0 in (0, 128)]
    atopWt = [wtile([128, AH], attnW[c0:c0 + 128, :], f"atopWt{c0}") for c0 in (0, 128)]
    abotWt = [wtile([dsz, AH], attnW[H + d0:H + d0 + dsz, :], f"abotWt{d0}")
              for (d0, dsz) in DCH]
    probWt = [wtile([128, 1], probW[c0:c0 + 128, :], f"probWt{c0}") for c0 in (0, 128)]
    W1t = [wtile([128, HLU], W1[h0:h0 + 128, :], f"W1t{h0}")
           for h0 in range(0, 3 * H, 128)]
    W2t = [wtile([128, NCLS], W2[m0:m0 + 128, :], f"W2t{m0}")
           for m0 in range(0, HLU, 128)]
    b1row = wtile([1, HLU], b1[:].rearrange("(o h) -> o h", o=1), "b1row")
    onesN = wp.tile([1, B], f32, name="onesN")
    nc.gpsimd.memset(onesN[:], 1.0)

    def col(src, h0, name):
        return wtile([128, 1], src[h0:h0 + 128].rearrange("(h o) -> h o", o=1), name)
    smfcb_c = [col(smfcb, h0, f"smfcb{h0}") for h0 in (0, 128)]
    ctxfcb_c = [col(ctxfcb, h0, f"ctxfcb{h0}") for h0 in (0, 128)]
    smb_c = [col(smb, c0, f"smbc{c0}") for c0 in (0, 128)]
    gamma_c = [col(gamma, m0, f"gammac{m0}") for m0 in range(0, HLU, 128)]
    beta_c = [col(beta, m0, f"betac{m0}") for m0 in range(0, HLU, 128)]

    def bcast_row(src, n, name):
        t = wp.tile([128, n], f32, name=name)
        nc.sync.dma_start(t[0:1, :], src[:].rearrange("(o c) -> o c", o=1))
        nc.gpsimd.partition_broadcast(t[:], t[0:1, :])
        return t
    ctxb_b = bcast_row(ctxb, C, "ctxb_b")
    ctxfcb_b = bcast_row(ctxfcb, H, "ctxfcb_b")
    attnb_b = bcast_row(attnb, AH, "attnb_b")
    b2_b = bcast_row(b2, NCLS, "b2_b")

    # ---------------- gather + norms + transposed activations ----------------
    def gather_side(idx_dram, name):
        tiles, rinv = [], []
        for m in range(4):
            it = sb.tile([128, 1], i32, name=f"{name}i{m}")
            nc.sync.dma_start(it[:], idx_dram[m * 128:(m + 1) * 128]
                              .rearrange("(p o) -> p o", o=1))
            st = sb.tile([128, D], f32, name=f"{name}s{m}")
            nc.gpsimd.indirect_dma_start(
                out=st[:], out_offset=None, in_=emb[:],
                in_offset=bass.IndirectOffsetOnAxis(ap=it[:, :1], axis=0))
            sq = sb.tile([128, D], f32, tag=f"{name}sq", bufs=2, name=f"{name}q{m}")
            nc.vector.tensor_tensor(sq[:], st[:], st[:], op=OP.mult)
            n2 = sb.tile([128, 1], f32, name=f"{name}n{m}")
            nc.vector.reduce_sum(n2[:], sq[:], axis=AX.X)
            nc.scalar.sqrt(n2[:], n2[:])
            rv = sb.tile([128, 1], f32, name=f"{name}r{m}")
            nc.vector.reciprocal(rv[:], n2[:])
            tiles.append(st)
            rinv.append(rv)
        stT = []
        for dc, (d0, dsz) in enumerate(DCH):
            tt = sb.tile([dsz, TP], f32, name=f"{name}T{dc}")
            for m in range(4):
                pt = ps.tile([128, 256], f32, tag="s", name=f"{name}tp{dc}_{m}")
                nc.tensor.transpose(pt[0:dsz, 0:128], tiles[m][:, d0:d0 + dsz],
                                    ident[:])
                nc.scalar.copy(tt[:, m * 128:(m + 1) * 128], pt[0:dsz, 0:128])
            stT.append(tt)
        return tiles, rinv, stT

    s1, rq, st1 = gather_side(q_idx, "q")
    s2, ra, st2 = gather_side(a_idx, "a")

    def dense_cols(stT_dc, L):
        # [d, TP] -> [d, BL, L] dense (b, t) view skipping the pad
        return stT_dc[:].rearrange("d (b p) -> d b p", b=BL)[:, :, 0:L]

    # dense (b, q) copy of sent1T for the attention matmul (stationary
    # operands must have a single free dim)
    st1d = []
    for dc, (d0, dsz) in enumerate(DCH):
        t = sb.tile([dsz, BL * QL], f32, name=f"st1d{dc}")
        nc.vector.tensor_copy(t[:].rearrange("d (b q) -> d b q", b=BL),
                              dense_cols(st1[dc], QL))
        st1d.append(t)

    # ---------------- sm-cnn branches (feature1 / feature2) ----------------
    fcT = [sb.tile([128, BL], f32, name=f"fcT{i}") for i in range(6)]

    def sm_branch(stT, L, out0, out1):
        nT = L - K + 1
        PT = []
        for m in range(6):
            pp = pz.tile([128, 512], f32, tag="z", name=f"smp{L}_{m}")
            for dc in range(3):
                nc.tensor.matmul(pp[:, 0:BL * L], smWt[dc][:, m * 128:(m + 1) * 128],
                                 dense_cols(stT[dc], L),
                                 start=(dc == 0), stop=(dc == 2))
            t = sb.tile([128, BL * L], f32, tag=f"smPT{L}", bufs=6,
                        name=f"smPT{L}_{m}")
            nc.scalar.copy(t[:], pp[:, 0:BL * L])
            PT.append(t)
        mres = []
        for h in range(2):
            def sh(mi, off):
                return PT[mi][:].rearrange("p (b t) -> p b t", b=BL)[:, :, off:off + nT]
            y = sb.tile([128, BL * nT], f32, tag=f"smy{L}", bufs=2, name=f"smy{L}_{h}")
            yv = y[:].rearrange("p (b t) -> p b t", b=BL)
            nc.vector.tensor_tensor(yv, sh(h, 0), sh(2 + h, 1), op=OP.add)
            nc.vector.tensor_tensor(yv, yv, sh(4 + h, 2), op=OP.add)
            mx = sb.tile([128, BL], f32, name=f"smm{L}_{h}")
            nc.vector.reduce_max(mx[:], yv, axis=AX.X)
            nc.scalar.activation(mx[:], mx[:], AF.Relu, bias=smb_c[h][:, :1])
            mres.append(mx)
        for hh, out in enumerate((out0, out1)):
            fp = ps.tile([128, 256], f32, tag="s", name=f"smf{L}_{hh}")
            for cc in range(2):
                nc.tensor.matmul(fp[:, 0:BL], smfcWt[cc][:, hh * 128:(hh + 1) * 128],
                                 mres[cc][:], start=(cc == 0), stop=(cc == 1))
            nc.scalar.activation(out[:], fp[:, 0:BL], AF.Identity,
                                 bias=smfcb_c[hh][:, :1])

    sm_branch(st1, QL, fcT[0], fcT[1])
    sm_branch(st2, AL, fcT[2], fcT[3])

    # ---------------- Z = sent2 @ ctxW staged to DRAM ----------------
    for m in range(4):
        zs = sb.tile([128, KC3], f32, tag="zs", bufs=4, name=f"zs{m}")
        for nn in range(2):
            zp2 = pz.tile([128, 512], f32, tag="z", name=f"zp{m}_{nn}")
            for dc in range(3):
                nc.tensor.matmul(zp2[:, 0:384],
                                 st2[dc][:, m * 128:(m + 1) * 128],
                                 ctxWt[dc][:, nn * 384:(nn + 1) * 384],
                                 start=(dc == 0), stop=(dc == 2))
            nc.scalar.copy(zs[:, nn * 384:(nn + 1) * 384], zp2[:, 0:384])
        nc.sync.dma_start(z_dram[:][m * 128:(m + 1) * 128, :], zs[:])

    # ---------------- cos per batch elem + staging of cos / G bands --------
    for b in range(BL):
        g, bb = b // 3, b % 3
        dp = ps.tile([128, 256], f32, tag="s", name=f"dots{b}")
        for dc in range(3):
            nc.tensor.matmul(dp[0:QL, 0:AL], st1[dc][:, b * PB:b * PB + QL],
                             st2[dc][:, b * PB:b * PB + AL],
                             start=(dc == 0), stop=(dc == 2))
        csb = sb.tile([QL, AL], f32, tag="csb", bufs=2, name=f"cs{b}")
        rqs = rq[b // 2][(b % 2) * PB:(b % 2) * PB + QL, 0:1]
        nc.vector.tensor_tensor(csb[:], dp[0:QL, 0:AL], rqs.to_broadcast([QL, AL]),
                                op=OP.mult)
        ctp = ps.tile([128, 256], f32, tag="s", name=f"ctp{b}")
        nc.tensor.transpose(ctp[0:AL, 0:QL], csb[:], ident[0:QL, 0:QL])
        cT = sb.tile([68, QL], f32, tag="cT", bufs=2, name=f"cT{b}")
        nc.gpsimd.memset(cT[:], 0.0)
        ras = ra[b // 2][(b % 2) * PB:(b % 2) * PB + AL, 0:1]
        nc.vector.tensor_scalar_mul(cT[0:AL, :], ctp[0:AL, 0:QL], ras[:, :1])
        nc.sync.dma_start(dview(cos_dram, b * 68 * 40, [[40, 68], [1, 40]]), cT[:])
        nc.sync.dma_start(
            dview(bd_dram[g], bb * 1240, [[3600, NCH], [120, 10], [1, QL]]),
            dview(cos_dram, b * 68 * 40, [[320, NCH], [40, 10], [1, QL]]))
        for k in range(K):
            nc.sync.dma_start(
                dview(g_dram[g], (bb * 10 + k) * RC,
                      [[30 * RC, NCH], [RC + C, TcH], [1, C]]),
                dview(z_dram, (b * PB + k) * KC3 + k * C,
                      [[TcH * KC3, NCH], [KC3, TcH], [1, C]]))

    # ---------------- banded-G matmuls, max, fc, attention ----------------
    feat_g, featT_g = [], []
    for g, (gb0, gs) in enumerate(GROUPS):
        Mg = gs * QL
        bdall = sb.tile([30, NCH * 120], f32, name=f"bdall{g}")
        nc.sync.dma_start(bdall[:].rearrange("p (j q) -> p j q", j=NCH),
                          dview(bd_dram[g], 0, [[120, 30], [3600, NCH], [1, 120]]))
        acc = sb.tile([120, C], f32, tag="acc", bufs=3, name=f"acc{g}")
        for j in range(NCH):
            last = (j == NCH - 1)
            nfree = 512 if last else RC
            gt = sb.tile([30, RC], f32, tag="gsb", bufs=3, name=f"g{g}_{j}")
            nc.sync.dma_start(gt[:, 0:nfree],
                              dview(g_dram[g], j * 30 * RC, [[RC, 30], [1, nfree]]))
            lhs = bdall[:, j * 120:j * 120 + Mg]
            if not last:
                gp1 = pg.tile([120, 1024], f32, tag="g", name=f"gp1_{g}_{j}")
                gp2 = pg.tile([120, 1024], f32, tag="g", name=f"gp2_{g}_{j}")
                for nn in range(2):
                    nc.tensor.matmul(gp1[0:Mg, nn * 512:(nn + 1) * 512], lhs,
                                     gt[:, nn * 512:(nn + 1) * 512],
                                     start=True, stop=True)
                for nn in range(2):
                    nc.tensor.matmul(gp2[0:Mg, nn * 512:(nn + 1) * 512], lhs,
                                     gt[:, 1024 + nn * 512:1024 + (nn + 1) * 512],
                                     start=True, stop=True)
                r1 = sb.tile([120, C], f32, tag="red", bufs=4, name=f"r1_{g}_{j}")
                r2 = sb.tile([120, C], f32, tag="red", bufs=4, name=f"r2_{g}_{j}")
                nc.vector.reduce_max(r1[0:Mg, :],
                                     gp1[0:Mg, :].rearrange("p (t c) -> p c t", t=4),
                                     axis=AX.X)
                nc.vector.reduce_max(r2[0:Mg, :],
                                     gp2[0:Mg, :].rearrange("p (t c) -> p c t", t=4),
                                     axis=AX.X)
                nc.vector.tensor_tensor(r1[0:Mg, :], r1[0:Mg, :], r2[0:Mg, :],
                                        op=OP.max)
                if j == 0:
                    nc.vector.tensor_copy(acc[0:Mg, :], r1[0:Mg, :])
                else:
                    nc.vector.tensor_tensor(acc[0:Mg, :], acc[0:Mg, :],
                                            r1[0:Mg, :], op=OP.max)
            else:
                gp1 = pg.tile([120, 1024], f32, tag="g", name=f"gp1_{g}_{j}")
                nc.tensor.matmul(gp1[0:Mg, 0:512], lhs, gt[:, 0:512],
                                 start=True, stop=True)
                r1 = sb.tile([120, C], f32, tag="red", bufs=4, name=f"r7_{g}")
                nc.vector.reduce_max(r1[0:Mg, :],
                                     gp1[0:Mg, 0:512].rearrange("p (t c) -> p c t", t=2),
                                     axis=AX.X)
                nc.vector.tensor_tensor(acc[0:Mg, :], acc[0:Mg, :], r1[0:Mg, :],
                                        op=OP.max)
        M3 = sb.tile([120, C], f32, tag="M3", bufs=3, name=f"M3_{g}")
        nc.vector.tensor_tensor(M3[0:Mg, :], acc[0:Mg, :], ctxb_b[0:Mg, :],
                                op=OP.add)
        nc.scalar.activation(M3[0:Mg, :], M3[0:Mg, :], AF.Relu)
        M3T = []
        for cc in range(2):
            tp = ps.tile([128, 256], f32, tag="s", name=f"m3t{g}_{cc}")
            nc.tensor.transpose(tp[:, 0:Mg], M3[0:Mg, cc * 128:(cc + 1) * 128],
                                ident[0:Mg, 0:Mg])
            t = sb.tile([128, 120], f32, tag="M3T", bufs=6, name=f"M3T{g}_{cc}")
            nc.scalar.copy(t[:, 0:Mg], tp[:, 0:Mg])
            M3T.append(t)
        fp = ps.tile([128, 256], f32, tag="s", name=f"fp{g}")
        for cc in range(2):
            nc.tensor.matmul(fp[0:Mg, 0:H], M3T[cc][:, 0:Mg], ctxfcWt[cc][:],
                             start=(cc == 0), stop=(cc == 1))
        feat = sb.tile([120, H], f32, tag="feat", bufs=3, name=f"feat{g}")
        nc.vector.tensor_tensor(feat[0:Mg, :], fp[0:Mg, 0:H], ctxfcb_b[0:Mg, :],
                                op=OP.add)
        feat_g.append(feat)
        featT = []
        for hh in range(2):
            ftp = ps.tile([128, 256], f32, tag="s", name=f"ftp{g}_{hh}")
            for cc in range(2):
                nc.tensor.matmul(ftp[:, 0:Mg], ctxfcWt[cc][:, hh * 128:(hh + 1) * 128],
                                 M3T[cc][:, 0:Mg], start=(cc == 0), stop=(cc == 1))
            t = sb.tile([128, 120], f32, tag="featT", bufs=6, name=f"featT{g}_{hh}")
            nc.scalar.activation(t[:, 0:Mg], ftp[:, 0:Mg], AF.Identity,
                                 bias=ctxfcb_c[hh][:, :1])
            featT.append(t)
        featT_g.append(featT)

    # ---------------- attention scores + softmax ----------------
    score_all = sb.tile([1, 8 * QL], f32, name="score_all")
    for g, (gb0, gs) in enumerate(GROUPS):
        Mg = gs * QL
        ap_ = ps.tile([128, 256], f32, tag="s", name=f"attn{g}")
        nc.tensor.matmul(ap_[0:Mg, 0:AH], featT_g[g][0][:, 0:Mg], atopWt[0][:],
                         start=True, stop=False)
        nc.tensor.matmul(ap_[0:Mg, 0:AH], featT_g[g][1][:, 0:Mg], atopWt[1][:],
                         start=False, stop=False)
        for dc in range(3):
            nc.tensor.matmul(ap_[0:Mg, 0:AH],
                             st1d[dc][:, g * 120:g * 120 + Mg], abotWt[dc][:],
                             start=False, stop=(dc == 2))
        T = sb.tile([120, AH], f32, tag="T", bufs=3, name=f"T{g}")
        nc.vector.tensor_tensor(T[0:Mg, :], ap_[0:Mg, 0:AH], attnb_b[0:Mg, :],
                                op=OP.add)
        nc.scalar.activation(T[0:Mg, :], T[0:Mg, :], AF.Tanh)
        scp = ps.tile([128, 256], f32, tag="s", name=f"sc{g}")
        for cc in range(2):
            ttp = ps.tile([128, 256], f32, tag="s", name=f"Tt{g}_{cc}")
            nc.tensor.transpose(ttp[:, 0:Mg], T[0:Mg, cc * 128:(cc + 1) * 128],
                                ident[0:Mg, 0:Mg])
            tts = sb.tile([128, 120], f32, tag="Tts", bufs=2, name=f"Tts{g}_{cc}")
            nc.scalar.copy(tts[:, 0:Mg], ttp[:, 0:Mg])
            nc.tensor.matmul(scp[0:1, 0:Mg], probWt[cc][:], tts[:, 0:Mg],
                             start=(cc == 0), stop=(cc == 1))
        nc.vector.tensor_copy(score_all[0:1, g * 120:g * 120 + Mg], scp[0:1, 0:Mg])

    sview = score_all[:].rearrange("p (b q) -> p b q", b=BL)
    smx = sb.tile([1, BL], f32, name="smx")
    nc.vector.reduce_max(smx[:], sview, axis=AX.X)
    sexp = sb.tile([1, 8 * QL], f32, name="sexp")
    nc.vector.tensor_tensor(sexp[:].rearrange("p (b q) -> p b q", b=BL), sview,
                            smx[:].rearrange("p (b o) -> p b o", o=1)
                            .to_broadcast([1, BL, QL]), op=OP.subtract)
    nc.scalar.activation(sexp[:], sexp[:], AF.Exp)
    ssum = sb.tile([1, BL], f32, name="ssum")
    nc.vector.reduce_sum(ssum[:], sexp[:].rearrange("p (b q) -> p b q", b=BL),
                         axis=AX.X)
    nc.vector.reciprocal(ssum[:], ssum[:])
    prob = sb.tile([1, 8 * QL], f32, name="prob")
    nc.vector.tensor_tensor(prob[:].rearrange("p (b q) -> p b q", b=BL),
                            sexp[:].rearrange("p (b q) -> p b q", b=BL),
                            ssum[:].rearrange("p (b o) -> p b o", o=1)
                            .to_broadcast([1, BL, QL]), op=OP.mult)
    nc.sync.dma_start(dview(prob_dram, 0, [[320, 1], [1, 320]]), prob[:])
    for g in range(3):
        nc.sync.dma_start(dview(pbd_dram, g * 360, [[121, 3], [3, QL]]),
                          dview(prob_dram, g * 120, [[QL, 3], [1, QL]]))

    # feature3^T = feat.T @ probBD
    for g, (gb0, gs) in enumerate(GROUPS):
        Mg = gs * QL
        pbd = sb.tile([120, 3], f32, tag="pbd", bufs=3, name=f"pbd{g}")
        nc.sync.dma_start(pbd[:], dview(pbd_dram, g * 360, [[3, 120], [1, 3]]))
        for cc in range(2):
            f3p = ps.tile([128, 256], f32, tag="s", name=f"f3{g}_{cc}")
            nc.tensor.matmul(f3p[:, 0:gs], feat_g[g][0:Mg, cc * 128:(cc + 1) * 128],
                             pbd[0:Mg, 0:gs], start=True, stop=True)
            nc.vector.tensor_copy(fcT[4 + cc][:, gb0:gb0 + gs], f3p[:, 0:gs])

    # ---------------- AllGather + replicated tail ----------------
    for hh in range(6):
        nc.sync.dma_start(fc_dram[:][hh * 128:(hh + 1) * 128, :], fcT[hh][:])
    nc.gpsimd.collective_compute(
        "AllGather", OP.bypass, replica_groups=[list(range(NCORES))],
        ins=[fc_dram[:].rearrange("a b -> (a b)")],
        outs=[fcall_dram[:]])

    fcall_sb = []
    for ch in range(6):
        r = sb.tile([128, B], f32, name=f"fcall{ch}")
        nc.sync.dma_start(
            r[:], dview(fcall_dram, ch * 128 * BL,
                        [[BL, 128], [3 * H * BL, NCORES], [1, BL]]))
        fcall_sb.append(r)

    foT = []
    for m in range(4):
        hp = ps.tile([128, 256], f32, tag="s", name=f"hp{m}")
        for ch in range(6):
            nc.tensor.matmul(hp[:, 0:B], W1t[ch][:, m * 128:(m + 1) * 128],
                             fcall_sb[ch][:], start=(ch == 0), stop=False)
        nc.tensor.matmul(hp[:, 0:B], b1row[0:1, m * 128:(m + 1) * 128], onesN[:],
                         start=False, stop=True)
        hT = sb.tile([128, B], f32, name=f"hT{m}")
        nc.scalar.copy(hT[:], hp[:, 0:B])
        s1t = sb.tile([128, 1], f32, name=f"bs1{m}")
        nc.vector.reduce_sum(s1t[:], hT[:], axis=AX.X)
        sqt = sb.tile([128, B], f32, tag="bsq", name=f"bsq{m}")
        nc.scalar.square(sqt[:], hT[:])
        s2t = sb.tile([128, 1], f32, name=f"bs2{m}")
        nc.vector.reduce_sum(s2t[:], sqt[:], axis=AX.X)
        nc.scalar.mul(s1t[:], s1t[:], 1.0 / B)
        nc.scalar.mul(s2t[:], s2t[:], 1.0 / B)
        musq = sb.tile([128, 1], f32, name=f"bmu2{m}")
        nc.vector.tensor_tensor(musq[:], s1t[:], s1t[:], op=OP.mult)
        nc.vector.tensor_tensor(s2t[:], s2t[:], musq[:], op=OP.subtract)
        nc.vector.tensor_scalar_add(s2t[:], s2t[:], EPS_BN)
        nc.scalar.sqrt(s2t[:], s2t[:])
        nc.vector.reciprocal(s2t[:], s2t[:])
        scl = sb.tile([128, 1], f32, name=f"bscl{m}")
        nc.vector.tensor_tensor(scl[:], gamma_c[m][:], s2t[:], op=OP.mult)
        sht = sb.tile([128, 1], f32, name=f"bsh{m}")
        nc.vector.tensor_tensor(sht[:], s1t[:], scl[:], op=OP.mult)
        nc.vector.tensor_tensor(sht[:], beta_c[m][:], sht[:], op=OP.subtract)
        ft = sb.tile([128, B], f32, name=f"foT{m}")
        nc.scalar.activation(ft[:], hT[:], AF.Tanh, bias=sht[:, :1],
                             scale=scl[:, :1])
        foT.append(ft)

    lgp = ps.tile([128, 256], f32, tag="s", name="lgp")
    for m in range(4):
        nc.tensor.matmul(lgp[0:B, 0:NCLS], foT[m][:], W2t[m][:],
                         start=(m == 0), stop=(m == 3))
    lg = sb.tile([B, NCLS], f32, name="lg")
    nc.vector.tensor_tensor(lg[:], lgp[0:B, 0:NCLS], b2_b[0:B, :], op=OP.add)
    lmx = sb.tile([B, 1], f32, name="lmx")
    nc.vector.reduce_max(lmx[:], lg[:], axis=AX.X)
    nc.vector.tensor_tensor(lg[:], lg[:], lmx[:, :1].to_broadcast([B, NCLS]),
                            op=OP.subtract)
    lex = sb.tile([B, NCLS], f32, name="lex")
    nc.scalar.activation(lex[:], lg[:], AF.Exp)
    lsum = sb.tile([B, 1], f32, name="lsum")
    nc.vector.reduce_sum(lsum[:], lex[:], axis=AX.X)
    nc.scalar.activation(lsum[:], lsum[:], AF.Ln)
    pr = sb.tile([B, NCLS], f32, name="pr")
    nc.vector.tensor_tensor(pr[:], lg[:], lsum[:, :1].to_broadcast([B, NCLS]),
                            op=OP.subtract)
    nc.sync.dma_start(preds_out[:], pr[:])

    fo = sb.tile([B, HLU], f32, name="fo")
    for m in range(4):
        fop = ps.tile([128, 256], f32, tag="s", name=f"fop{m}")
        nc.tensor.transpose(fop[0:B, 0:128], foT[m][:], ident[:])
        nc.scalar.copy(fo[:, m * 128:(m + 1) * 128], fop[0:B, 0:128])
    nc.sync.dma_start(feat_out[:], fo[:])


def build(n_cores=NCORES):
    nc = bacc.Bacc("TRN2", target_bir_lowering=False, debug=False,
                   num_devices=n_cores)
    with tile.TileContext(nc) as tc:
        with ExitStack() as ctx:
            _emit(nc, tc, ctx)
    nc.compile()
    return nc


_CACHE = {}


def make_in_maps(question, answer, embedding,
                 sm_convW, sm_convb, sm_fcW, sm_fcb,
                 ctx_convW, ctx_convb, ctx_fcW, ctx_fcb,
                 attnW, attnb, probW, W1, b1, gamma, beta, W2, b2,
                 n_cores=NCORES):
    question = np.asarray(question, dtype=np.int32)
    answer = np.asarray(answer, dtype=np.int32)
    wmap = {
        "emb": np.ascontiguousarray(np.asarray(embedding, np.float32)),
        "smW": np.ascontiguousarray(np.asarray(sm_convW, np.float32)),
        "smb": np.asarray(sm_convb, np.float32),
        "smfcW": np.ascontiguousarray(np.asarray(sm_fcW, np.float32)),
        "smfcb": np.asarray(sm_fcb, np.float32),
        "ctxW": np.ascontiguousarray(np.asarray(ctx_convW, np.float32)),
        "ctxb": np.asarray(ctx_convb, np.float32),
        "ctxfcW": np.ascontiguousarray(np.asarray(ctx_fcW, np.float32)),
        "ctxfcb": np.asarray(ctx_fcb, np.float32),
        "attnW": np.ascontiguousarray(np.asarray(attnW, np.float32)),
        "attnb": np.asarray(attnb, np.float32),
        "probW": np.ascontiguousarray(np.asarray(probW, np.float32)),
        "W1": np.ascontiguousarray(np.asarray(W1, np.float32)),
        "b1": np.asarray(b1, np.float32),
        "gamma": np.asarray(gamma, np.float32),
        "beta": np.asarray(beta, np.float32),
        "W2": np.ascontiguousarray(np.asarray(W2, np.float32)),
        "b2": np.asarray(b2, np.float32),
    }
    in_maps = []
    for c in range(n_cores):
        qp = np.zeros((BL, PB), np.int32)
        ap_ = np.zeros((BL, PB), np.int32)
        qp[:, :QL] = question[c * BL:(c + 1) * BL]
        ap_[:, :AL] = answer[c * BL:(c + 1) * BL]
        m = dict(wmap)
        m["q_idx"] = qp.reshape(-1)
        m["a_idx"] = ap_.reshape(-1)
        in_maps.append(m)
    return in_maps


def kernel(question, answer, ext_feats, embedding,
           sm_convW, sm_convb, sm_fcW, sm_fcb,
           ctx_convW, ctx_convb, ctx_fcW, ctx_fcb,
           attnW, attnb, probW, W1, b1, gamma, beta, W2, b2):
    from concourse.bass_utils import run_bass_kernel_spmd
    if "nc" not in _CACHE:
        _CACHE["nc"] = build(NCORES)
    nc = _CACHE["nc"]
    in_maps = make_in_maps(question, answer, embedding,
                           sm_convW, sm_convb, sm_fcW, sm_fcb,
                           ctx_convW, ctx_convb, ctx_fcW, ctx_fcb,
                           attnW, attnb, probW, W1, b1, gamma, beta, W2, b2)
    res = run_bass_kernel_spmd(nc, in_maps, list(range(NCORES)))
    preds = np.asarray(res.results[0]["preds"], np.float32)
    feat = np.asarray(res.results[0]["feat"], np.float32)
    return preds, feat
